# revision 9
# baseline (speedup 1.0000x reference)
"""Multi-head attention (B=256, T=256, H=6, D=64, C=384) on 8 TRN2 NeuronCores.

bf16 redesign of the fp32r baseline. Data-parallel over batch: each core owns
32 batch elements, weights replicated, no collectives.

Gains over fp32r baseline:
  - all matmuls bf16 (1 cyc/col at ANY N; fp32r needs N>=256, and its 191ns
    self-loading LDWEIGHTS hiccups; bf16 LDW is 95ns and fully hidden)
  - causal split: scores s1 computes only t'>=128 (N=128 at full rate);
    AV right half accumulates chunk b with N=128 matmuls; pb left block
    (all-masked) never computed, so no Pool zerofill
  - single merged exp per pair (768 cols) instead of 2 ACT calls
  - elementwise ops emit bf16 (halves SBUF traffic), output DMA'd as bf16

Per-batch stage structure and slot pipeline retained from the baseline
(3 head-pair slots; stages offset by batch so the PE never idles):

  stage        PE work                         drained by
  qk_p         6 mm N=256 -> qk_ps (1 bank)    ACT/DVE copy -> bf16
  scores_p     s0 N=256 + s1 N=128 per side,   ACT exp (1 call, 768 cols)
               sides row-paired (K=64)         Pool triangle masks x2
  AV_p         6 mm N=128 accumulate           DVE recip + 2 TT mult -> bf16
               [v|ones]^T @ p
  v_s          3 mm N=384 (bf16)               ACT copy s0 / DVE copy s1
  proj_t       3 mm N=384 (bf16)               ACT bias-add -> bf16, DMA

PSUM (8 banks): w 2 (1/side), qk x2, o x2, v x1, out x1.
"""

import numpy as np
import ml_dtypes

import concourse.bacc as bacc
import concourse.mybir as mybir
import concourse.tile as tile
from concourse.bass_utils import run_bass_kernel_spmd

F32 = mybir.dt.float32
BF16 = mybir.dt.bfloat16
AF = mybir.ActivationFunctionType

N_CORES = 8
B, T, C = 256, 256, 384
H, D = 6, 64
B_LOC = B // N_CORES  # 32
SCALE = 1.0 / float(np.sqrt(np.float32(C)))


def build_nc(b_loc=B_LOC):
    nc = bacc.Bacc("TRN2", target_bir_lowering=False, debug=False)

    xt_d = nc.dram_tensor("xt", [b_loc, C, T], BF16, kind="ExternalInput")
    wqk_d = nc.dram_tensor("wqk", [3, 128, 768], BF16, kind="ExternalInput")
    wv_d = nc.dram_tensor("wv", [3, 128, 384], BF16, kind="ExternalInput")
    wp_d = nc.dram_tensor("wp", [3, 128, 384], BF16, kind="ExternalInput")
    bias_d = nc.dram_tensor("bias", [128, 384], F32, kind="ExternalInput")
    out_d = nc.dram_tensor("out", [b_loc, T, C], BF16, kind="ExternalOutput")

    with tile.TileContext(nc) as tc:
        with (
            tc.tile_pool(name="const", bufs=1) as cpool,
            tc.tile_pool(name="xt", bufs=3) as xt_pool,
            tc.tile_pool(name="qksb", bufs=6) as qksb_pool,
            tc.tile_pool(name="p", bufs=4) as p_pool,
            tc.tile_pool(name="osb", bufs=6) as osb_pool,
            tc.tile_pool(name="outsb", bufs=3) as outsb_pool,
            tc.tile_pool(name="rsb", bufs=2) as rsb_pool,
            tc.tile_pool(name="pw", bufs=1, space="PSUM") as pw,
            tc.tile_pool(name="pqk", bufs=2, space="PSUM") as pqk,
            tc.tile_pool(name="po", bufs=2, space="PSUM") as po,
            tc.tile_pool(name="pv", bufs=1, space="PSUM") as pv,
            tc.tile_pool(name="pout", bufs=1, space="PSUM") as pout,
        ):
            # xt(0) gates the first matmuls together with wqk: stripe its
            # three C-chunks across the three DMA queues ahead of the wqk
            # chunks so both land ~1.2us earlier.
            xt0_early = xt_pool.tile([128, 3, 256], BF16, tag="xt", name="xt_sb")
            x0ap = xt_d.ap()[0].rearrange("(k p) t -> k p t", p=128)
            nc.sync.dma_start(xt0_early[:, 0, :], x0ap[0])
            nc.scalar.dma_start(xt0_early[:, 1, :], x0ap[1])
            nc.gpsimd.dma_start(xt0_early[:, 2, :], x0ap[2])
            wqk = cpool.tile([128, 3, 768], BF16)
            wv = cpool.tile([128, 3, 384], BF16)
            wp = cpool.tile([128, 3, 384], BF16)
            bias = cpool.tile([128, 384], F32)
            # wqk gates the first qk matmul: split its 3 chunks across three
            # DMA queues so it lands in ~1/3 the serialized time. xt(0) is
            # queued first on sync (first consumer); wp follows wqk-c2 on
            # gpsimd (not needed until the first proj, ~10us in).
            nc.sync.dma_start(wqk[:, 0, :], wqk_d.ap()[0])
            nc.scalar.dma_start(wqk[:, 1, :], wqk_d.ap()[1])
            nc.gpsimd.dma_start(wqk[:, 2, :], wqk_d.ap()[2])
            nc.scalar.dma_start(wv[:], wv_d.ap().rearrange("k p m -> p k m"))
            nc.gpsimd.dma_start(wp[:], wp_d.ap().rearrange("k p m -> p k m"))
            nc.scalar.dma_start(bias[:], bias_d.ap())

            # v_aug ring: [v_h (64) | ones (64)] per head; ones written once.
            v_ring = []
            for r in range(4):
                v_aug = cpool.tile([128, 6, 128], BF16, name=f"v_aug{r}")
                nc.gpsimd.memset(v_aug[:, :, 64:128], 1.0)
                v_ring.append(v_aug)

            xt_t, qk_sb, p_sb, o_sb = {}, {}, {}, {}

            def e_dma_xt(b):
                xt = xt_pool.tile([128, 3, 256], BF16, tag="xt", name="xt_sb")
                nc.sync.dma_start(xt[:], xt_d.ap()[b].rearrange("(k p) t -> p k t", p=128))
                xt_t[b] = xt

            qk_ps_t = {}

            def e_qk_mm(b, p):
                qk_ps = pqk.tile([128, 512], F32, tag="qk", name="qk_ps")
                for qk in range(2):
                    m = (p * 2 + qk) * 128
                    for k in range(3):
                        nc.tensor.matmul(
                            qk_ps[:, qk * 256:(qk + 1) * 256],
                            wqk[:, k, m:m + 128],
                            xt_t[b][:, k, :],
                            start=(k == 0), stop=(k == 2),
                        )
                qk_ps_t[(b, p)] = qk_ps

            def e_qk_copy(b, p):
                # PSUM f32 -> SBUF bf16. Alternate engines to balance load:
                # pair 1 on DVE, pairs 0/2 on ACT.
                sb = qksb_pool.tile([128, 512], BF16, tag="qksb", name="qk_sb_t")
                src = qk_ps_t.pop((b, p))
                nc.scalar.activation(sb[:], src[:], AF.Copy)
                qk_sb[(b, p)] = sb

            def e_scores(b, p):
                # w[s, t'] = q_s . k_t' per side; sides in separate banks so
                # the K=64 row-paired matmuls (rows 0:64 / 64:128) overlap.
                # Layout per side: cols 0:256 = s-chunk0 (t' 0:256),
                # cols 256:384 = s-chunk1 right half (t' 128:256).
                w = pw.tile([128, 2, 512], F32, tag="w", name="w_ps")
                qsb = qk_sb.pop((b, p))
                # s1 (short, N=128) first: its pair drains faster, shrinking
                # the window the scheduler fills with foreign full-row mms.
                for sc, (qlo, qhi, klo, khi, wlo, whi) in enumerate(
                    ((128, 256, 384, 512, 256, 384), (0, 128, 256, 512, 0, 256))
                ):
                    for side in range(2):
                        lo = side * 64
                        nc.tensor.matmul(
                            w[:, side, wlo:whi],
                            qsb[lo:lo + 64, qlo:qhi],
                            qsb[lo:lo + 64, klo:khi],
                            start=True, stop=True,
                        )
                pt = p_pool.tile([128, 2, 384], BF16, tag="p", name="p_t")
                nc.scalar.activation(pt[:], w[:, :, 0:384], AF.Exp, scale=SCALE)
                # causal triangles (keep t' >= s) on the two diagonal blocks
                nc.gpsimd.affine_select(
                    out=pt[:, :, 0:128], in_=pt[:, :, 0:128],
                    compare_op=mybir.AluOpType.is_ge, fill=0.0,
                    base=0, pattern=[[0, 2], [1, 128]], channel_multiplier=-1,
                )
                nc.gpsimd.affine_select(
                    out=pt[:, :, 256:384], in_=pt[:, :, 256:384],
                    compare_op=mybir.AluOpType.is_ge, fill=0.0,
                    base=0, pattern=[[0, 2], [1, 128]], channel_multiplier=-1,
                )
                p_sb[(b, p)] = pt

            def e_v(b, s):
                v_ps = pv.tile([128, 384], F32, tag="v", name="v_ps")
                for k in range(3):
                    nc.tensor.matmul(
                        v_ps[:],
                        xt_t[b][:, k, s * 128:(s + 1) * 128],
                        wv[:, k, :],
                        start=(k == 0), stop=(k == 2),
                    )
                v_aug = v_ring[(2 * b + s) % 4]
                nc.scalar.activation(
                    v_aug[:, :, 0:64],
                    v_ps[:].rearrange("p (h d) -> p h d", h=6),
                    AF.Copy,
                )

            def e_av(b, p):
                o_ps = po.tile([128, 512], F32, tag="o", name="o_ps")
                pt = p_sb.pop((b, p))
                va = v_ring[(2 * b) % 4]
                vb = v_ring[(2 * b + 1) % 4]
                for side in range(2):
                    h = 2 * p + side
                    base = side * 256
                    # single N=256 va matmul; left half's group never sees a
                    # stop (no-op on HW), checker bypassed
                    nc.tensor.matmul(o_ps[:, base:base + 256], va[:, h, :],
                                     pt[:, side, 0:256], start=True, stop=False,
                                     skip_group_check=True)
                    nc.tensor.matmul(o_ps[:, base + 128:base + 256], vb[:, h, :],
                                     pt[:, side, 256:384], start=False, stop=True,
                                     skip_group_check=True)
                sb = osb_pool.tile([128, 256], BF16, tag="osb", name="o_sb_t")
                r = rsb_pool.tile([128, 512], F32, tag="r", name="r_sb")
                # custom DVE ops ignore partition offsets on HW: full range
                nc.vector.reciprocal_approx_fast(out=r[:], in_=o_ps[:])
                nc.vector.tensor_tensor(sb[0:64, :], o_ps[0:64, 0:256],
                                        r[64:128, 0:256], mybir.AluOpType.mult)
                nc.vector.tensor_tensor(sb[64:128, :], o_ps[0:64, 256:512],
                                        r[64:128, 256:512], mybir.AluOpType.mult)
                o_sb[(b, p)] = sb

            def e_proj(b, t):
                out_ps = pout.tile([128, 384], F32, tag="out", name="out_ps")
                for ch in range(3):
                    nc.tensor.matmul(
                        out_ps[:],
                        o_sb[(b, ch)][:, t * 128:(t + 1) * 128],
                        wp[:, ch, :],
                        start=(ch == 0), stop=(ch == 2),
                    )
                if t == 1:
                    for ch in range(3):
                        del o_sb[(b, ch)]
                out_sb = outsb_pool.tile([128, 384], BF16, tag="outsb", name="out_sb_t")
                nc.vector.tensor_tensor(out_sb[:], out_ps[:], bias[:], mybir.AluOpType.add)
                nc.sync.dma_start(out_d.ap()[b, t * 128:(t + 1) * 128, :], out_sb[:])

            def g(b):
                return 0 <= b < b_loc

            xt_t[0] = xt0_early
            e_dma_xt(1)
            for i in range(-2, b_loc + 1):
                # slot0  (qk_copy leads e_scores so the copy for batch i+2
                # runs ahead of the exp in the ACT queue: the next slot's
                # scores-LDWEIGHTS wait on this copy, the exp has a spare slot)
                if g(i + 2): e_qk_mm(i + 2, 0)
                if g(i + 2): e_qk_copy(i + 2, 0)
                if g(i + 1): e_scores(i + 1, 0)
                if g(i):     e_av(i, 1)
                # slot1
                if g(i + 2): e_qk_mm(i + 2, 1)
                if g(i + 2): e_qk_copy(i + 2, 1)
                if g(i + 1): e_scores(i + 1, 1)
                if g(i - 1): e_proj(i - 1, 1)
                if g(i):     e_av(i, 2)
                if g(i + 2): e_v(i + 2, 0)
                # slot2
                if g(i + 2): e_qk_mm(i + 2, 2)
                if g(i + 2): e_qk_copy(i + 2, 2)
                if g(i + 1): e_scores(i + 1, 2)
                if g(i + 1): e_av(i + 1, 0)
                if g(i + 2): e_v(i + 2, 1)
                if g(i):     e_proj(i, 0)
                if g(i + 4): e_dma_xt(i + 4)

    nc.compile()
    return nc


def _host_prep(x, wk, wq, wv, wproj, bproj):
    """Build the per-core input maps (host-side shard + repack + bf16 cast)."""
    x = np.ascontiguousarray(x, dtype=np.float32)
    wk = np.asarray(wk, dtype=np.float32)
    wq = np.asarray(wq, dtype=np.float32)
    wv = np.asarray(wv, dtype=np.float32)
    wproj = np.asarray(wproj, dtype=np.float32)
    bproj = np.asarray(bproj, dtype=np.float32)

    # packed q/k weights: [pair, q/k, C, 128] -> [chunk(3), 128, 768]
    wqp = wq.reshape(3, 2, C, D)
    wkp = wk.reshape(3, 2, C, D)
    qk = np.empty((3, 2, C, 128), dtype=np.float32)
    qk[:, 0, :, 0:64] = wqp[:, 0]
    qk[:, 0, :, 64:128] = wqp[:, 1]
    qk[:, 1, :, 0:64] = wkp[:, 0]
    qk[:, 1, :, 64:128] = wkp[:, 1]
    wqk_h = np.ascontiguousarray(
        qk.transpose(2, 0, 1, 3).reshape(3, 128, 768)).astype(ml_dtypes.bfloat16)
    wv_h = np.ascontiguousarray(
        wv.transpose(1, 0, 2).reshape(C, H * D).reshape(3, 128, 384)).astype(ml_dtypes.bfloat16)
    wp_h = np.ascontiguousarray(wproj.reshape(3, 128, 384)).astype(ml_dtypes.bfloat16)
    bias_h = np.ascontiguousarray(
        np.broadcast_to(bproj.reshape(1, 384), (128, 384)), dtype=np.float32)

    in_maps = []
    for c in range(N_CORES):
        xs = x[c * B_LOC:(c + 1) * B_LOC]  # [B_LOC, T, C]
        xt = np.ascontiguousarray(xs.transpose(0, 2, 1)).astype(ml_dtypes.bfloat16)
        in_maps.append({
            "xt": xt, "wqk": wqk_h, "wv": wv_h, "wp": wp_h, "bias": bias_h,
        })
    return in_maps


_NC_CACHE = {}


def run(inputs, trace=False, **kw):
    """Run on the 8 NeuronCores; returns (output, BassKernelResults)."""
    if "nc" not in _NC_CACHE:
        _NC_CACHE["nc"] = build_nc()
    nc = _NC_CACHE["nc"]
    in_maps = _host_prep(
        inputs["x"], inputs["wk"], inputs["wq"], inputs["wv"],
        inputs["wproj"], inputs["bproj"],
    )
    res = run_bass_kernel_spmd(nc, in_maps, core_ids=list(range(N_CORES)),
                               trace=trace, **kw)
    out = np.concatenate(
        [res.results[c]["out"].astype(np.float32) for c in range(N_CORES)], axis=0)
    return out, res


def kernel(**inputs):
    inputs = {k: np.asarray(v, dtype=np.float32) for k, v in inputs.items()}
    out, _ = run(inputs, trace=False)
    return out
